# revision 61
# baseline (speedup 1.0000x reference)
"""Trainium2 Bass kernel: multi-head self-attention with RoPE + sigmoid gating.

Computes, for fixed shapes B=2, S=2048, E=1024, H=16, D=64:
    qkv = x @ w_qkv ; q,k roped (concatenated-halves layout)
    att = softmax(q k^T / sqrt(D)) ; out = (att @ v * sigmoid(x @ w_gate)) @ w_out + b_out

Sharding: 8 cores = 2 (batch) x 4 (head groups of 4 heads).  Each core computes a
row-parallel partial of the output projection for its batch (its 4 heads' slice of
the E contraction); the host sums the 4 partials per batch and adds b_out.

v6: decoupled PSUM pools + PE cost-law exploitation.
  - measured PE cost law: matmul ~ N*min(K,M)/128 cycles + ~40-70ns; adjacent
    matmuls on disjoint partition (row/col) groups fuse in the PE array
    (second completes in ~4ns), so PV runs as M=64 head pairs and the four
    M=1 denominator matmuls are emitted adjacently to 4-fuse across column
    groups.  This halves PV vs the v2 ones-column (M=65->128) layout.
  - the attention phase is elementwise-bound (exp on ACT/DVE), so the scores
    pipeline gets a dedicated double-buffered PSUM pool; filler groups
    (gate/oproj/recB) run from a separate 1-bank pool and cannot inject
    their ACT/DVE round-trips into the scores->exp chain.
  - exp split 21 ACT (exact) / 11 DVE (Schraudolph bit-trick) per chunk, at
    most one DVE unit per sk-step so the engines run concurrently; the
    denominator normalization cancels the correlated part of the bt error,
    so rel err stays ~1.7e-2 at this split.
  - reciprocal broadcast via K=64 one-hot matmuls (half the K=128 cost).
  - gate projection spread over chunks 0-2 (chunk 0 was PE-bound).
  - per-k input tiles + interleaved DMA order for an early PE start.

All matmuls bf16 (inputs pre-cast host-side); PSUM accumulation fp32.
Softmax skips max-subtraction (scores*scale are O(+-6)).

Device-side layouts (per core; host preps/permutes/casts all of these):
    xn_k [128, 1024]   x[b]^T cols 0:1024, contraction tile k (8 tiles)
    xm_k [128, 1024]   x[b]^T cols 1024:2048
    wqk_k [128, 512]   columns: [q_even | q_odd | k_even | k_odd], each 128 = 4 heads x 32
    wv   [128, KT, 256] v columns for the 4 heads (natural order)
    wg   [128, KT, 256] w_gate columns for the 4 heads' output dims
    wo   [128, 2, 1024] w_out rows for the 4 heads, as 2 pair-tiles of 128
    cs/sn [128, 2048]  cos/sin RoPE tables, rows = 4x32 freqs, cols = position
    hot  [128, 512]    hot[32h, 128h + r] = 1 (recB broadcast selector)
Output: out [2048, 1024] bf16 partial (no bias).
"""

import numpy as np
import ml_dtypes

B, S, E, H, D = 2, 2048, 1024, 16, 64
HC = 4            # heads per core
NCORES = 8
KT = E // 128     # 8 contraction tiles
ST = S // 128     # 16 sequence tiles
SQ = 512          # attention sq chunk
NCH = S // SQ     # 4 chunks
ROPE_THETA = 10000.0

# exp-unit engine split: (sk, g) units in this set go to DVE via the
# bit-trick; the rest run exact exp on ScalarE.  At most ONE unit per
# sk-step (ACT and DVE run concurrently, never alternating) and 3-step
# spacing within each g so the per-g scores->exp->scores loop stays slack.
BT_UNITS = ({(sk, 0) for sk in (2, 5, 8, 11, 14)} |
            {(sk, 1) for sk in (0, 3, 6, 9, 12, 15)})

_CACHE = {}
PE_LOG = []   # labels of matmuls in emission order (devloop diagnostics)

# Results of the most recent kernel() call, for test harnesses.
LAST_RESULTS = None


# ---------------------------------------------------------------------------
# BIR postprocess: the walrus build in this image accepts only ONE sync-wait
# command per lowered TPB instruction (Drain/NoOp/LDWEIGHTS/...).  Tile emits
# instructions with several waits; split the excess onto preceding single-wait
# NoOps on the same engine (program order preserves the blocking semantics).
# Installed by patching concourse's compile_bir_kernel in this process.
# ---------------------------------------------------------------------------

def _split_waits(bir_bytes, limit=1):
    import json as _json
    m = _json.loads(bir_bytes)
    counter = [0]

    def fix_block(instrs):
        out = []
        for ins in instrs:
            w = ins.get("sync_info", {}).get("on_wait", [])
            if len(w) > limit:
                chunks = [w[i:i + limit] for i in range(0, len(w), limit)]
                ins["sync_info"]["on_wait"] = chunks[-1]
                for ch in chunks[:-1]:
                    counter[0] += 1
                    out.append({
                        "name": f"I-waitsplit-{counter[0]}",
                        "opcode": "NoOp",
                        "engine": ins.get("engine"),
                        "ins": [],
                        "outs": [],
                        "sync_info": {"on_update": [], "on_wait": ch},
                    })
            out.append(ins)
        return out

    def walk(d):
        if isinstance(d, dict):
            for k, v in d.items():
                if k == "instructions" and isinstance(v, list):
                    d[k] = fix_block(v)
                else:
                    walk(v)
        elif isinstance(d, list):
            for v in d:
                walk(v)

    walk(m)
    return _json.dumps(m).encode()


def _install_birfix():
    if _CACHE.get("birfix"):
        return
    _CACHE["birfix"] = True
    import concourse.bass_utils as bu
    import concourse.bass2jax as b2j

    orig = bu.compile_bir_kernel

    def patched(bir_json, tmpdir, neff_name="file.neff"):
        return orig(_split_waits(bir_json), tmpdir, neff_name=neff_name)

    bu.compile_bir_kernel = patched
    b2j.compile_bir_kernel = patched


def _build_nc():
    import concourse.bass as bass
    import concourse.mybir as mybir
    from concourse.tile import TileContext

    bf = mybir.dt.bfloat16
    f32 = mybir.dt.float32
    i16 = mybir.dt.int16
    MUL = mybir.AluOpType.mult
    SUB = mybir.AluOpType.subtract
    ADD = mybir.AluOpType.add
    Act = mybir.ActivationFunctionType

    scale = float(D) ** -0.5
    # bit-trick exp constants: bf16 bits of e^(scale*s) ~ 128*(scale*log2e*s
    # + 126.94269504)
    BT_A = float(scale * np.log2(np.e) * 128.0)
    BT_B = float(126.94269504 * 128.0)

    nc = bass.Bass()
    PE_LOG.clear()

    def MM(label, *args, **kw):
        PE_LOG.append(label)
        nc.tensor.matmul(*args, **kw)

    xn_d = [nc.dram_tensor(f"xn{k}", (128, 1024), bf, kind="ExternalInput")
            for k in range(KT)]
    xm_d = [nc.dram_tensor(f"xm{k}", (128, 1024), bf, kind="ExternalInput")
            for k in range(KT)]
    wqk_d = [nc.dram_tensor(f"wqk{k}", (128, 512), bf, kind="ExternalInput")
             for k in range(KT)]
    wv_d = nc.dram_tensor("wv", (E, HC * 64), bf, kind="ExternalInput")
    wg_d = nc.dram_tensor("wg", (E, HC * 64), bf, kind="ExternalInput")
    wo_d = nc.dram_tensor("wo", (128, 2, E), bf, kind="ExternalInput")
    cs_d = nc.dram_tensor("cs", (128, S), bf, kind="ExternalInput")
    sn_d = nc.dram_tensor("sn", (128, S), bf, kind="ExternalInput")
    hot_d = nc.dram_tensor("hot", (128, 512), bf, kind="ExternalInput")
    out_d = nc.dram_tensor("out", (S, E), bf, kind="ExternalOutput")

    with TileContext(nc) as tc:
        with (
            tc.tile_pool(name="const", bufs=1) as cpool,
            tc.tile_pool(name="big", bufs=1) as bpool,
            tc.tile_pool(name="work", bufs=3) as wpool,
            tc.tile_pool(name="expool", bufs=10) as expool,
            tc.tile_pool(name="sct", bufs=2, space="PSUM") as sctpool,
            tc.tile_pool(name="fil", bufs=1, space="PSUM") as filpool,
            tc.tile_pool(name="pvp", bufs=1, space="PSUM") as pvpool,
            tc.tile_pool(name="dnp", bufs=1, space="PSUM") as dnpool,
            tc.tile_pool(name="scr", bufs=2, space="DRAM") as scrpool,
        ):
            # ---- weights/x in, interleaved so the first qkproj group can
            # start as soon as wqk_0 + xn_0 land ----
            wqks = []
            xns = []
            xms = []
            for k in range(KT):
                w = cpool.tile([128, 512], bf, tag=f"wqk{k}", name=f"wqk{k}")
                nc.sync.dma_start(w, wqk_d[k][:, :])
                wqks.append(w)
                t = cpool.tile([128, 1024], bf, tag=f"xn{k}", name=f"xn{k}")
                nc.sync.dma_start(t, xn_d[k][:, :])
                xns.append(t)
            cs = cpool.tile([128, S], bf)
            nc.sync.dma_start(cs, cs_d[:, :])
            sn = cpool.tile([128, S], bf)
            nc.sync.dma_start(sn, sn_d[:, :])
            for k in range(KT):
                t = cpool.tile([128, 1024], bf, tag=f"xm{k}", name=f"xm{k}")
                nc.sync.dma_start(t, xm_d[k][:, :])
                xms.append(t)
            wv = cpool.tile([128, KT, 256], bf)
            nc.sync.dma_start(wv, wv_d[:, :].rearrange("(k p) m -> p k m", p=128))
            wg = cpool.tile([128, KT, 256], bf)
            nc.sync.dma_start(wg, wg_d[:, :].rearrange("(k p) m -> p k m", p=128))
            wo = cpool.tile([128, 2, E], bf)
            nc.sync.dma_start(wo, wo_d[:, :, :])
            # one-hot rows for the K=64 recB broadcast: hot[32h, 128h+r] = 1;
            # heads 0,1 live in rows 0:64, heads 2,3 in rows 64:128
            hot = cpool.tile([128, 512], bf)
            nc.sync.dma_start(hot, hot_d[:, :])

            ones32 = cpool.tile([128, 32], bf)
            nc.vector.memset(ones32, 1.0)
            for _ in range(2):
                rb0 = wpool.tile([128, SQ], bf, tag="rec_bf", name="rb_init", bufs=2)
                nc.gpsimd.memset(rb0, 1.0)

            # warm the ACT exp/tanh table set before it is on the critical path
            warm = cpool.tile([1, 8], f32)
            nc.vector.memset(warm, 0.0)
            nc.scalar.activation(warm, warm, Act.Exp)

            def xslice(k, o, w, rh=None):
                """x^T tile k, columns [o, o+w); rh selects a 64-row half."""
                rs = slice(0, 128) if rh is None else slice(64 * rh, 64 * rh + 64)
                if o < 1024:
                    assert o + w <= 1024
                    return xns[k][rs, o:o + w]
                return xms[k][rs, o - 1024:o - 1024 + w]

            # ---- phase A: qk projection (4 M-tiles) + RoPE, per 1024-col slice ----
            qkraw = [bpool.tile([128, S], bf, tag=f"qkraw{m}", name=f"qkraw{m}") for m in range(4)]
            qTop = bpool.tile([128, S], bf)
            qBot = bpool.tile([128, S], bf)
            kTop = bpool.tile([128, S], bf)
            kBot = bpool.tile([128, S], bf)
            qR = [bpool.tile([128, S], bf, tag=f"qR{g}", name=f"qR{g}") for g in range(2)]
            kR = [bpool.tile([128, S], bf, tag=f"kR{g}", name=f"kR{g}") for g in range(2)]
            for n2 in range(S // 1024):
                sl = slice(n2 * 1024, (n2 + 1) * 1024)
                for m in range(4):
                    # contraction split into 64-row halves accumulating into
                    # TWO separate psum tiles: adjacent top/bot matmuls are on
                    # disjoint row groups and fuse in the PE array (~216ns for
                    # the pair vs 264 for one full-K matmul).  Separate tiles
                    # avoid the same-tile interleaved-region accumulation that
                    # hangs the device.  One DVE add replaces the psum copy.
                    psA = sctpool.tile([128, 1024], f32, tag="sct", name="ps_qkA")
                    psB = sctpool.tile([128, 1024], f32, tag="sct", name="ps_qkB")
                    for half in range(2):
                        o = 512 * half
                        for k in range(KT):
                            for rh in range(2):
                                ps = psA if rh == 0 else psB
                                MM(f"qkproj n{n2} m{m} k{k} r{rh}",
                                    ps[:, o:o + 512],
                                    lhsT=wqks[k][64 * rh:64 * rh + 64,
                                                 m * 128:(m + 1) * 128],
                                    rhs=xslice(k, n2 * 1024 + o, 512, rh),
                                    start=(k == 0), stop=(k == KT - 1),
                                )
                    # DVE cannot read two PSUM operands: stage psA in SBUF
                    # (ACT copy, same cost as the old psum->sbuf move)
                    qka = wpool.tile([128, 1024], f32, tag="qka", name="qka", bufs=2)
                    nc.scalar.copy(qka, psA)
                    nc.vector.tensor_tensor(qkraw[m][:, sl], qka, psB, ADD)
                # RoPE for this 1024-column slice (full 128-partition ops)
                for (ev, od, top, bot) in ((qkraw[0], qkraw[1], qTop, qBot),
                                           (qkraw[2], qkraw[3], kTop, kBot)):
                    t1 = wpool.tile([128, 1024], bf, tag="rt1", name="rt1")
                    t2 = wpool.tile([128, 1024], bf, tag="rt2", name="rt2")
                    nc.vector.tensor_tensor(t1, ev[:, sl], cs[:, sl], MUL)
                    nc.vector.tensor_tensor(t2, od[:, sl], sn[:, sl], MUL)
                    nc.vector.tensor_tensor(top[:, sl], t1, t2, SUB)
                    t3 = wpool.tile([128, 1024], bf, tag="rt1", name="rt3")
                    t4 = wpool.tile([128, 1024], bf, tag="rt2", name="rt4")
                    nc.vector.tensor_tensor(t3, ev[:, sl], sn[:, sl], MUL)
                    nc.vector.tensor_tensor(t4, od[:, sl], cs[:, sl], MUL)
                    nc.vector.tensor_tensor(bot[:, sl], t3, t4, ADD)
                # assemble per-pair roped tensors for this slice
                # qR[g] rows: [64*h2 + j] j<32: top of head 2g+h2 ; j>=32: bottom
                for g in range(2):
                    for (top, bot, dst) in ((qTop, qBot, qR[g]), (kTop, kBot, kR[g])):
                        for h2 in range(2):
                            h = 2 * g + h2
                            nc.sync.dma_start(dst[64 * h2:64 * h2 + 32, sl],
                                              top[32 * h:32 * h + 32, sl])
                            nc.sync.dma_start(dst[64 * h2 + 32:64 * h2 + 64, sl],
                                              bot[32 * h:32 * h + 32, sl])

            # ---- phase B: v projection into [128, 4, 64] stationary tiles ----
            vos = []
            for s in range(ST):
                vo = bpool.tile([128, HC, 64], bf, tag=f"vo{s}", name=f"vo{s}")
                vos.append(vo)
                ps = sctpool.tile([128, 1024], f32, tag="sct", name="ps_v")
                for k in range(KT):
                    MM(f"vproj s{s} k{k}",
                        ps[:, :256],
                        lhsT=xslice(k, s * 128, 128),
                        rhs=wv[:, k, :],
                        start=(k == 0), stop=(k == KT - 1),
                    )
                nc.scalar.copy(vo.rearrange("p h w -> p (h w)"), ps[:, :256])

            # ---- attention with interleaved fillers ----
            gP = [bpool.tile([128, S], bf, tag=f"gP{g}", name=f"gP{g}") for g in range(2)]
            ag = [bpool.tile([128, S], bf, tag=f"ag{g}", name=f"ag{g}") for g in range(2)]

            def gate_group(g, half2):
                # half a gate-projection column-tile: 8 matmuls + tanh + affine
                def emit():
                    o = half2 * 512
                    sl = slice(o, o + 512)
                    ps = filpool.tile([128, 512], f32, tag="fil", name="ps_g")
                    for k in range(KT):
                        MM(f"gate g{g} o{o} k{k}",
                            ps[:, 0:512],
                            lhsT=wg[:, k, g * 128:(g + 1) * 128],
                            rhs=xslice(k, o, 512),
                            start=(k == 0), stop=(k == KT - 1),
                        )
                    th = wpool.tile([128, 512], bf, tag="th", name="th")
                    nc.scalar.activation(th, ps[:, 0:512], Act.Tanh, scale=0.5)
                    # sigmoid(x) = 0.5*tanh(x/2) + 0.5  (gpsimd: SBUF only)
                    nc.gpsimd.tensor_scalar(gP[g][:, sl], th, 0.5, 0.5, MUL, ADD)
                return emit

            def fil_tile(tail):
                # in-chunk fillers use the 1-bank pool; tail fillers run when
                # the scores pipeline is done, so they use the big sct pool
                if tail:
                    return sctpool.tile([128, 1024], f32, tag="sct", name="ps_tail")
                return filpool.tile([128, 512], f32, tag="fil", name="ps_fil")

            def recb_pair(c, rec_bf, recBs, g, cols=slice(0, 512), tail=False):
                # broadcast 1/den for the head pair g into ONE [128, 512]
                # tile: rows 0:64 <- rec_{2g}, rows 64:128 <- rec_{2g+1}.
                # K=64, M=64 per matmul; the two MMs fuse in the PE array.
                def emit():
                    ps = fil_tile(tail)
                    recBs[g] = ps
                    ksl = slice(0, 64) if g == 0 else slice(64, 128)
                    for h2 in range(2):
                        h = 2 * g + h2
                        MM(f"recB c{c} h{h}",
                            ps[64 * h2:64 * (h2 + 1), cols],
                            lhsT=hot[ksl, 128 * h:128 * h + 64],
                            rhs=rec_bf[ksl, cols],
                            start=True, stop=True,
                        )
                return emit

            def norm_pair(c, recBs, uus, g, half=None):
                # after recB: agh = uu * recB ; ag = agh * gate  (DVE + gpsimd)
                def emit():
                    csl = slice(c * SQ, (c + 1) * SQ)
                    if c == NCH - 1:
                        # uu is already gated: one mul to normalized output
                        cw = slice(half * 256, half * 256 + 256) if half is not None \
                            else slice(0, 512)
                        nc.vector.tensor_tensor(
                            ag[g][:, c * SQ + cw.start:c * SQ + cw.stop],
                            uus[g][:, cw],
                            recBs[g][:, cw], MUL)
                        return
                    agh = wpool.tile([128, SQ], f32, tag=f"agh{g}", name="agh", bufs=1)
                    nc.vector.tensor_tensor(agh, uus[g], recBs[g][:, 0:512], MUL)
                    nc.gpsimd.tensor_tensor(ag[g][:, csl], agh, gP[g][:, csl], MUL)
                return emit

            def oproj_half(c, st, n, tail=False):
                # one 512-col half of the output projection for row-tile st
                def emit():
                    s = (SQ // 128) * c + st
                    ps = fil_tile(tail)
                    for g in range(2):
                        MM(f"oproj c{c} st{st} n{n} g{g}",
                            ps[:, 0:512],
                            lhsT=ag[g][:, s * 128:(s + 1) * 128],
                            rhs=wo[:, g, n * 512:(n + 1) * 512],
                            start=(g == 0), stop=(g == 1),
                        )
                    ob = wpool.tile([128, 512], bf, tag="ob", name="ob", bufs=3)
                    # tail: split copies between ACT and DVE (both run the
                    # normalize chain; a single engine serializes the tail)
                    if tail and (st + n + c) % 2 == 1:
                        nc.scalar.copy(ob, ps[:, 0:512])
                    else:
                        nc.vector.tensor_copy(ob, ps[:, 0:512])
                    nc.sync.dma_start(out_d[s * 128:(s + 1) * 128,
                                            n * 512:(n + 1) * 512], ob)
                emit.deferred = (c == NCH - 2)
                return emit

            # filler queues: emitted between sk-steps of each chunk.
            # gate half2=c is needed by norm_pair(c) early in chunk c+1, so
            # halves 0,1 go in chunk 0 and 2,3 spread into chunks 1,2 (the
            # ACT-paced chunks have ~2 gate groups of PE slack each).
            fillers = {c: [] for c in range(NCH + 1)}
            fillers[0] = [gate_group(g, h2) for h2 in range(2) for g in range(2)]
            fillers[1] = [gate_group(g, 2) for g in range(2)]
            fillers[2] = [gate_group(g, 3) for g in range(2)]

            norm_state = {}
            for c in range(NCH):
                csl = slice(c * SQ, (c + 1) * SQ)
                pvp = [pvpool.tile([128, SQ], f32, tag=f"pvp{g}", name=f"pvp{g}")
                       for g in range(2)]
                denp = dnpool.tile([128, SQ], f32, tag="denp", name="denp")
                exs = {}

                def scores_step(sk):
                    for g in range(2):
                        sct = sctpool.tile([128, 1024], f32, tag="sct", name="sct")
                        for h2 in range(2):
                            MM(f"score c{c} sk{sk} g{g} h{h2}",
                                sct[:, h2 * 512:(h2 + 1) * 512],
                                lhsT=kR[g][64 * h2:64 * (h2 + 1), sk * 128:(sk + 1) * 128],
                                rhs=qR[g][64 * h2:64 * (h2 + 1), csl],
                                start=True, stop=True,
                            )
                        ex = expool.tile([128, 1024], bf, tag="ex", name="ex")
                        if (sk, g) in BT_UNITS:
                            # Schraudolph bit-trick on DVE: int16 bits of bf16
                            nc.vector.tensor_scalar(
                                ex.bitcast(i16), sct, BT_A, BT_B, MUL, ADD)
                        else:
                            nc.scalar.activation(ex, sct, Act.Exp, scale=scale)
                        exs[(sk, g)] = ex

                def pv_step(sk):
                    # pv pairs first, then all four den MMs adjacently so the
                    # M=1 quads fuse across the four PE column groups
                    exg = [exs.pop((sk, 0)), exs.pop((sk, 1))]
                    for g in range(2):
                        for h2 in range(2):
                            h = 2 * g + h2
                            MM(f"pv c{c} sk{sk} h{h}",
                                pvp[g][64 * h2:64 * (h2 + 1), :],
                                lhsT=vos[sk][:, h, :],
                                rhs=exg[g][:, h2 * 512:(h2 + 1) * 512],
                                start=(sk == 0), stop=(sk == ST - 1),
                            )
                    for h in range(HC):
                        g, h2 = divmod(h, 2)
                        MM(f"den c{c} sk{sk} h{h}",
                            denp[32 * h:32 * h + 1, :],
                            lhsT=ones32[:, 0:1],
                            rhs=exg[g][:, h2 * 512:(h2 + 1) * 512],
                            start=(sk == 0), stop=(sk == ST - 1),
                            tile_position=(0, 32 * h),
                        )

                fq = fillers[c]
                for sk in range(ST):
                    scores_step(sk)
                    if sk > 2:
                        pv_step(sk - 3)
                    if fq and sk >= 5:
                        fq.pop(0)()
                for sk in (ST - 3, ST - 2, ST - 1):
                    pv_step(sk)
                while fq:
                    fq.pop(0)()

                # ---- normalize: free pv banks early, recip off critical path ----
                last = (c == NCH - 1)
                rec_bf = wpool.tile([128, SQ], bf, tag="rec_bf", name="rec_bf", bufs=2)

                if last:
                    # latency-critical tail: dstack + two 256-wide DVE waves,
                    # dstack copies on ACT (idle in the tail)
                    dstack = wpool.tile([128, SQ], f32, tag="dstack", name="dstack", bufs=1)
                    nc.gpsimd.memset(dstack, 1.0)
                    for h in range(HC):
                        nc.scalar.copy(dstack[32 * h:32 * h + 1, :],
                                       denp[32 * h:32 * h + 1, :])
                    rec128 = wpool.tile([128, SQ], f32, tag="rec128", name="rec128", bufs=1)
                    nc.vector.reciprocal(out=rec128[:, 0:256],
                                         in_=dstack[:, 0:256])
                    nc.vector.tensor_copy(rec_bf[:, 0:256], rec128[:, 0:256])
                    nc.vector.reciprocal(out=rec128[:, 256:512],
                                         in_=dstack[:, 256:512])
                    nc.scalar.copy(rec_bf[:, 256:512], rec128[:, 256:512])
                else:
                    # only 4x512 of the [128,512] den tile is live, so pack the
                    # four rows into [128,16] via SBUF DMA (off-engine), run the
                    # reciprocal on 16 elems/lane (~0.3us instead of 3.4us of
                    # DVE), and scatter back.  This unclogs the chunk-boundary
                    # DVE queue that was stalling the next chunk's bt-exps.
                    den_sb = wpool.tile([128, SQ], f32, tag="den_sb", name="den_sb", bufs=2)
                    nc.scalar.copy(den_sb, denp)
                    # free->partition reshape is illegal for SBUF-SBUF DMA, so
                    # bounce through scratch DRAM (linear; arbitrary APs)
                    den_scr = scrpool.tile([4, 512], f32, tag="den_scr", name="den_scr")
                    for h in range(HC):
                        nc.sync.dma_start(den_scr[h:h + 1, :],
                                          den_sb[32 * h:32 * h + 1, :])
                    dpack = wpool.tile([128, 16], f32, tag="dpack", name="dpack", bufs=2)
                    nc.sync.dma_start(
                        dpack, den_scr.rearrange("h (i j) -> (h i) j", j=16))
                    rpack = wpool.tile([128, 16], f32, tag="rpack", name="rpack", bufs=2)
                    nc.vector.reciprocal(out=rpack, in_=dpack)
                    rpack_bf = wpool.tile([128, 16], bf, tag="rpack_bf", name="rpack_bf", bufs=2)
                    nc.gpsimd.tensor_copy(rpack_bf, rpack)
                    rec_scr = scrpool.tile([128, 16], bf, tag="rec_scr", name="rec_scr")
                    nc.sync.dma_start(rec_scr, rpack_bf)
                    rs4 = rec_scr.rearrange("(h i) j -> h (i j)", h=4)
                    for h in range(HC):
                        nc.sync.dma_start(rec_bf[32 * h:32 * h + 1, :],
                                          rs4[h:h + 1, :])
                uus = []
                for g in range(2):
                    uu = wpool.tile([128, SQ], f32, tag=f"uu{g}", name=f"uu{g}", bufs=1)
                    if last:
                        # gated u: fold the gate multiply in here so it
                        # overlaps the reciprocal instead of trailing recB
                        nc.vector.tensor_tensor(uu, pvp[g], gP[g][:, csl], MUL)
                    else:
                        nc.vector.tensor_copy(uu, pvp[g])
                    uus.append(uu)

                recBs = [None, None]
                if c == NCH - 1:
                    # two waves: [recb+norm(A), st0, st1] then [recb+norm(B), ...]
                    for half in range(2):
                        cw = slice(half * 256, half * 256 + 256)
                        for g in range(2):
                            fillers[c + 1].append(
                                recb_pair(c, rec_bf, recBs, g, cw, tail=True))
                            fillers[c + 1].append(
                                norm_pair(c, recBs, uus, g, half))
                        for st in (2 * half, 2 * half + 1):
                            fillers[c + 1].append(oproj_half(c, st, 0, tail=True))
                            fillers[c + 1].append(oproj_half(c, st, 1, tail=True))
                else:
                    for g in range(2):
                        fillers[c + 1].append(recb_pair(c, rec_bf, recBs, g))
                        fillers[c + 1].append(norm_pair(c, recBs, uus, g))
                    for st in range(SQ // 128):
                        # the last chunk's predecessor defers three oproj
                        # tiles into the tail to cover the final normalize
                        tail = (c == NCH - 2)
                        dst = c + 2 if tail else c + 1
                        fillers[dst].append(oproj_half(c, st, 0, tail=tail))
                        fillers[dst].append(oproj_half(c, st, 1, tail=tail))

            # drain: deferred oproj halves lead (they cover the final
            # normalize/reciprocal chain), interleaved 2:1 with the waves
            deferred = [f for f in fillers[NCH] if getattr(f, "deferred", False)]
            waves = [f for f in fillers[NCH] if not getattr(f, "deferred", False)]
            drain = []
            while deferred or waves:
                for _ in range(2):
                    if deferred:
                        drain.append(deferred.pop(0))
                if waves:
                    drain.append(waves.pop(0))
            for f in drain:
                f()

    return nc


def _host_inputs(x, w_qkv, w_gate, w_out):
    """Build the 8 per-core input maps (all device tensors bf16)."""
    bf = ml_dtypes.bfloat16
    x = np.asarray(x, dtype=np.float32)
    w_qkv = np.asarray(w_qkv, dtype=np.float32)
    w_gate = np.asarray(w_gate, dtype=np.float32)
    w_out = np.asarray(w_out, dtype=np.float32)

    inv = 1.0 / (ROPE_THETA ** (np.arange(0, D, 2, dtype=np.float64) / D))   # [32]
    ang = np.arange(S, dtype=np.float64)[None, :] * inv[:, None]             # [32, S]
    cs = np.tile(np.cos(ang), (4, 1)).astype(bf)                             # [128, S]
    sn = np.tile(np.sin(ang), (4, 1)).astype(bf)

    hot = np.zeros((128, 512), dtype=bf)
    for h in range(HC):
        hot[32 * h, 128 * h:128 * (h + 1)] = 1.0

    wq = w_qkv[:, 0:E]
    wk = w_qkv[:, E:2 * E]
    wvv = w_qkv[:, 2 * E:3 * E]

    in_maps = []
    for c in range(NCORES):
        b = c // 4
        hs = HC * (c % 4)
        cols_ev = np.concatenate([(hs + h) * 64 + np.arange(0, 64, 2) for h in range(HC)])
        cols_od = cols_ev + 1
        wqk_p = np.concatenate(
            [wq[:, cols_ev], wq[:, cols_od], wk[:, cols_ev], wk[:, cols_od]], axis=1)
        vcols = np.concatenate([(hs + h) * 64 + np.arange(64) for h in range(HC)])
        wo_p = w_out[vcols, :].reshape(2, 128, E).transpose(1, 0, 2)
        xT = np.ascontiguousarray(x[b].T).astype(bf)
        m = {
            "wv": np.ascontiguousarray(wvv[:, vcols]).astype(bf),
            "wg": np.ascontiguousarray(w_gate[:, vcols]).astype(bf),
            "wo": np.ascontiguousarray(wo_p).astype(bf),
            "cs": cs,
            "sn": sn,
            "hot": hot,
        }
        for k in range(KT):
            m[f"wqk{k}"] = np.ascontiguousarray(wqk_p[k * 128:(k + 1) * 128, :]).astype(bf)
            m[f"xn{k}"] = np.ascontiguousarray(xT[k * 128:(k + 1) * 128, 0:1024])
            m[f"xm{k}"] = np.ascontiguousarray(xT[k * 128:(k + 1) * 128, 1024:2048])
        in_maps.append(m)
    return in_maps


def kernel(x, w_qkv, w_gate, w_out, b_out, n_heads):
    global LAST_RESULTS
    assert int(n_heads) == H
    x = np.asarray(x)
    assert x.shape == (B, S, E)

    from concourse.bass_utils import run_bass_kernel_spmd

    _install_birfix()
    if "nc" not in _CACHE:
        _CACHE["nc"] = _build_nc()
    nc = _CACHE["nc"]

    in_maps = _host_inputs(x, w_qkv, w_gate, w_out)
    import os
    trace = bool(int(os.environ.get("KERNEL_TRACE", "0")))
    tmpdir = os.environ.get("KERNEL_TRACE_DIR") if trace else None
    res = run_bass_kernel_spmd(nc, in_maps, list(range(NCORES)), trace=trace,
                               tmpdir=tmpdir)
    LAST_RESULTS = res

    out = np.zeros((B, S, E), dtype=np.float32)
    for c in range(NCORES):
        out[c // 4] += np.asarray(res.results[c]["out"], dtype=np.float32)
    out += np.asarray(b_out, dtype=np.float32)[None, None, :]
    return out


# revision 66
# speedup vs baseline: 1.0778x; 1.0778x over previous
"""Trainium2 Bass kernel: multi-head self-attention with RoPE + sigmoid gating.

Computes, for fixed shapes B=2, S=2048, E=1024, H=16, D=64:
    qkv = x @ w_qkv ; q,k roped (concatenated-halves layout)
    att = softmax(q k^T / sqrt(D)) ; out = (att @ v * sigmoid(x @ w_gate)) @ w_out + b_out

Sharding: 8 cores = 2 (batch) x 4 (head groups of 4 heads).  Each core computes a
row-parallel partial of the output projection for its batch (its 4 heads' slice of
the E contraction); the host sums the 4 partials per batch and adds b_out.

v6: decoupled PSUM pools + PE cost-law exploitation.
  - measured PE cost law: matmul ~ N*min(K,M)/128 cycles + ~40-70ns; adjacent
    matmuls on disjoint partition (row/col) groups fuse in the PE array
    (second completes in ~4ns), so PV runs as M=64 head pairs and the four
    M=1 denominator matmuls are emitted adjacently to 4-fuse across column
    groups.  This halves PV vs the v2 ones-column (M=65->128) layout.
  - the attention phase is elementwise-bound (exp on ACT/DVE), so the scores
    pipeline gets a dedicated double-buffered PSUM pool; filler groups
    (gate/oproj/recB) run from a separate 1-bank pool and cannot inject
    their ACT/DVE round-trips into the scores->exp chain.
  - exp split 21 ACT (exact) / 11 DVE (Schraudolph bit-trick) per chunk, at
    most one DVE unit per sk-step so the engines run concurrently; the
    denominator normalization cancels the correlated part of the bt error,
    so rel err stays ~1.7e-2 at this split.
  - reciprocal broadcast via K=64 one-hot matmuls (half the K=128 cost).
  - gate projection spread over chunks 0-2 (chunk 0 was PE-bound).
  - per-k input tiles + interleaved DMA order for an early PE start.

All matmuls bf16 (inputs pre-cast host-side); PSUM accumulation fp32.
Softmax skips max-subtraction (scores*scale are O(+-6)).

Device-side layouts (per core; host preps/permutes/casts all of these):
    xn_k [128, 1024]   x[b]^T cols 0:1024, contraction tile k (8 tiles)
    xm_k [128, 1024]   x[b]^T cols 1024:2048
    wqk_k [128, 512]   columns: [q_even | q_odd | k_even | k_odd], each 128 = 4 heads x 32
    wv   [128, KT, 256] v columns for the 4 heads (natural order)
    wg   [128, KT, 256] w_gate columns for the 4 heads' output dims
    wo   [128, 2, 1024] w_out rows for the 4 heads, as 2 pair-tiles of 128
    cs/sn [128, 2048]  cos/sin RoPE tables, rows = 4x32 freqs, cols = position
    hot  [128, 512]    hot[32h, 128h + r] = 1 (recB broadcast selector)
Output: out [2048, 1024] bf16 partial (no bias).
"""

import numpy as np
import ml_dtypes

B, S, E, H, D = 2, 2048, 1024, 16, 64
HC = 4            # heads per core
NCORES = 8
KT = E // 128     # 8 contraction tiles
ST = S // 128     # 16 sequence tiles
SQ = 512          # attention sq chunk
NCH = S // SQ     # 4 chunks
ROPE_THETA = 10000.0

# exp-unit engine split: (sk, g) units in this set go to DVE via the
# bit-trick; the rest run exact exp on ScalarE.  At most ONE unit per
# sk-step (ACT and DVE run concurrently, never alternating) and 3-step
# spacing within each g so the per-g scores->exp->scores loop stays slack.
BT_UNITS = ({(sk, 0) for sk in (2, 5, 8, 11, 14)} |
            {(sk, 1) for sk in (0, 3, 6, 9, 12, 15)})

_CACHE = {}
PE_LOG = []   # labels of matmuls in emission order (devloop diagnostics)

# Results of the most recent kernel() call, for test harnesses.
LAST_RESULTS = None


# ---------------------------------------------------------------------------
# BIR postprocess: the walrus build in this image accepts only ONE sync-wait
# command per lowered TPB instruction (Drain/NoOp/LDWEIGHTS/...).  Tile emits
# instructions with several waits; split the excess onto preceding single-wait
# NoOps on the same engine (program order preserves the blocking semantics).
# Installed by patching concourse's compile_bir_kernel in this process.
# ---------------------------------------------------------------------------

def _split_waits(bir_bytes, limit=1):
    import json as _json
    m = _json.loads(bir_bytes)
    counter = [0]

    def fix_block(instrs):
        out = []
        for ins in instrs:
            w = ins.get("sync_info", {}).get("on_wait", [])
            if len(w) > limit:
                chunks = [w[i:i + limit] for i in range(0, len(w), limit)]
                ins["sync_info"]["on_wait"] = chunks[-1]
                for ch in chunks[:-1]:
                    counter[0] += 1
                    out.append({
                        "name": f"I-waitsplit-{counter[0]}",
                        "opcode": "NoOp",
                        "engine": ins.get("engine"),
                        "ins": [],
                        "outs": [],
                        "sync_info": {"on_update": [], "on_wait": ch},
                    })
            out.append(ins)
        return out

    def walk(d):
        if isinstance(d, dict):
            for k, v in d.items():
                if k == "instructions" and isinstance(v, list):
                    d[k] = fix_block(v)
                else:
                    walk(v)
        elif isinstance(d, list):
            for v in d:
                walk(v)

    walk(m)
    return _json.dumps(m).encode()


def _install_birfix():
    if _CACHE.get("birfix"):
        return
    _CACHE["birfix"] = True
    import concourse.bass_utils as bu
    import concourse.bass2jax as b2j

    orig = bu.compile_bir_kernel

    def patched(bir_json, tmpdir, neff_name="file.neff"):
        return orig(_split_waits(bir_json), tmpdir, neff_name=neff_name)

    bu.compile_bir_kernel = patched
    b2j.compile_bir_kernel = patched


def _build_nc():
    import concourse.bass as bass
    import concourse.mybir as mybir
    from concourse.tile import TileContext

    bf = mybir.dt.bfloat16
    f32 = mybir.dt.float32
    i16 = mybir.dt.int16
    MUL = mybir.AluOpType.mult
    SUB = mybir.AluOpType.subtract
    ADD = mybir.AluOpType.add
    Act = mybir.ActivationFunctionType

    scale = float(D) ** -0.5
    # bit-trick exp constants: bf16 bits of e^(scale*s) ~ 128*(scale*log2e*s
    # + 126.94269504)
    BT_A = float(scale * np.log2(np.e) * 128.0)
    BT_B = float(126.94269504 * 128.0)

    nc = bass.Bass()
    PE_LOG.clear()

    def MM(label, *args, **kw):
        PE_LOG.append(label)
        nc.tensor.matmul(*args, **kw)

    xn_d = [nc.dram_tensor(f"xn{k}", (128, 1024), bf, kind="ExternalInput")
            for k in range(KT)]
    xm_d = [nc.dram_tensor(f"xm{k}", (128, 1024), bf, kind="ExternalInput")
            for k in range(KT)]
    wqka_d = [nc.dram_tensor(f"wqka{k}", (128, 256), bf, kind="ExternalInput")
              for k in range(KT)]
    wqkb_d = [nc.dram_tensor(f"wqkb{k}", (128, 256), bf, kind="ExternalInput")
              for k in range(KT)]
    wv_d = nc.dram_tensor("wv", (E, HC * 64), bf, kind="ExternalInput")
    wg_d = nc.dram_tensor("wg", (E, HC * 64), bf, kind="ExternalInput")
    wo_d = nc.dram_tensor("wo", (128, 2, E), bf, kind="ExternalInput")
    cs_d = nc.dram_tensor("cs", (128, S), bf, kind="ExternalInput")
    sn_d = nc.dram_tensor("sn", (128, S), bf, kind="ExternalInput")
    hot_d = nc.dram_tensor("hot", (128, 512), bf, kind="ExternalInput")
    out_d = nc.dram_tensor("out", (S, E), bf, kind="ExternalOutput")

    with TileContext(nc) as tc:
        with (
            tc.tile_pool(name="const", bufs=1) as cpool,
            tc.tile_pool(name="big", bufs=1) as bpool,
            tc.tile_pool(name="work", bufs=3) as wpool,
            tc.tile_pool(name="expool", bufs=10) as expool,
            tc.tile_pool(name="sct", bufs=2, space="PSUM") as sctpool,
            tc.tile_pool(name="fil", bufs=1, space="PSUM") as filpool,
            tc.tile_pool(name="pvp", bufs=1, space="PSUM") as pvpool,
            tc.tile_pool(name="dnp", bufs=1, space="PSUM") as dnpool,
            tc.tile_pool(name="scr", bufs=2, space="DRAM") as scrpool,
        ):
            # ---- weights/x in, interleaved so the first qkproj group can
            # start as soon as wqk_0 + xn_0 land ----
            wqkas = []
            wqkbs = []
            xns = []
            xms = []
            for k in range(KT):
                w = cpool.tile([128, 256], bf, tag=f"wqka{k}", name=f"wqka{k}")
                nc.sync.dma_start(w, wqka_d[k][:, :])
                wqkas.append(w)
                t = cpool.tile([128, 1024], bf, tag=f"xn{k}", name=f"xn{k}")
                nc.sync.dma_start(t, xn_d[k][:, :])
                xns.append(t)
            for k in range(KT):
                w = cpool.tile([128, 256], bf, tag=f"wqkb{k}", name=f"wqkb{k}")
                nc.sync.dma_start(w, wqkb_d[k][:, :])
                wqkbs.append(w)
            cs = cpool.tile([128, S], bf)
            nc.sync.dma_start(cs, cs_d[:, :])
            sn = cpool.tile([128, S], bf)
            nc.sync.dma_start(sn, sn_d[:, :])
            for k in range(KT):
                t = cpool.tile([128, 1024], bf, tag=f"xm{k}", name=f"xm{k}")
                nc.sync.dma_start(t, xm_d[k][:, :])
                xms.append(t)
            wv = cpool.tile([128, KT, 256], bf)
            nc.sync.dma_start(wv, wv_d[:, :].rearrange("(k p) m -> p k m", p=128))
            wg = cpool.tile([128, KT, 256], bf)
            nc.sync.dma_start(wg, wg_d[:, :].rearrange("(k p) m -> p k m", p=128))
            wo = cpool.tile([128, 2, E], bf)
            nc.sync.dma_start(wo, wo_d[:, :, :])
            # one-hot rows for the K=64 recB broadcast: hot[32h, 128h+r] = 1;
            # heads 0,1 live in rows 0:64, heads 2,3 in rows 64:128
            hot = cpool.tile([128, 512], bf)
            nc.sync.dma_start(hot, hot_d[:, :])

            ones32 = cpool.tile([128, 32], bf)
            nc.vector.memset(ones32, 1.0)
            for _ in range(2):
                rb0 = wpool.tile([128, SQ], bf, tag="rec_bf", name="rb_init", bufs=2)
                nc.gpsimd.memset(rb0, 1.0)

            # warm the ACT exp/tanh table set before it is on the critical path
            warm = cpool.tile([1, 8], f32)
            nc.vector.memset(warm, 0.0)
            nc.scalar.activation(warm, warm, Act.Exp)

            def xslice(k, o, w, rh=None):
                """x^T tile k, columns [o, o+w); rh selects a 64-row half."""
                rs = slice(0, 128) if rh is None else slice(64 * rh, 64 * rh + 64)
                if o < 1024:
                    assert o + w <= 1024
                    return xns[k][rs, o:o + w]
                return xms[k][rs, o - 1024:o - 1024 + w]

            # ---- phase A: qk projection (4 M-tiles) + RoPE, per 1024-col slice ----
            qkraw = [bpool.tile([128, S], bf, tag=f"qkraw{m}", name=f"qkraw{m}") for m in range(4)]
            qTop = bpool.tile([128, S], bf)
            qBot = bpool.tile([128, S], bf)
            kTop = bpool.tile([128, S], bf)
            kBot = bpool.tile([128, S], bf)
            qR = [bpool.tile([128, S], bf, tag=f"qR{g}", name=f"qR{g}") for g in range(2)]
            kR = [bpool.tile([128, S], bf, tag=f"kR{g}", name=f"kR{g}") for g in range(2)]
            for n2 in range(S // 1024):
                sl = slice(n2 * 1024, (n2 + 1) * 1024)
                for m in range(4):
                    ps = sctpool.tile([128, 1024], f32, tag="sct", name="ps_qk")
                    for half in range(2):
                        o = 512 * half
                        for k in range(KT):
                            MM(f"qkproj n{n2} m{m} k{k}",
                                ps[:, o:o + 512],
                                lhsT=(wqkas[k][:, m * 128:(m + 1) * 128] if m < 2
                                      else wqkbs[k][:, (m - 2) * 128:(m - 1) * 128]),
                                rhs=xslice(k, n2 * 1024 + o, 512),
                                start=(k == 0), stop=(k == KT - 1),
                            )
                    # ScalarE moves psum -> sbuf (keeps DVE free for RoPE)
                    nc.scalar.copy(qkraw[m][:, sl], ps)
                # RoPE for this 1024-column slice (full 128-partition ops)
                for (ev, od, top, bot) in ((qkraw[0], qkraw[1], qTop, qBot),
                                           (qkraw[2], qkraw[3], kTop, kBot)):
                    t1 = wpool.tile([128, 1024], bf, tag="rt1", name="rt1")
                    t2 = wpool.tile([128, 1024], bf, tag="rt2", name="rt2")
                    nc.vector.tensor_tensor(t1, ev[:, sl], cs[:, sl], MUL)
                    nc.vector.tensor_tensor(t2, od[:, sl], sn[:, sl], MUL)
                    nc.vector.tensor_tensor(top[:, sl], t1, t2, SUB)
                    t3 = wpool.tile([128, 1024], bf, tag="rt1", name="rt3")
                    t4 = wpool.tile([128, 1024], bf, tag="rt2", name="rt4")
                    nc.vector.tensor_tensor(t3, ev[:, sl], sn[:, sl], MUL)
                    nc.vector.tensor_tensor(t4, od[:, sl], cs[:, sl], MUL)
                    nc.vector.tensor_tensor(bot[:, sl], t3, t4, ADD)
                # assemble per-pair roped tensors for this slice
                # qR[g] rows: [64*h2 + j] j<32: top of head 2g+h2 ; j>=32: bottom
                for g in range(2):
                    for (top, bot, dst) in ((qTop, qBot, qR[g]), (kTop, kBot, kR[g])):
                        for h2 in range(2):
                            h = 2 * g + h2
                            nc.sync.dma_start(dst[64 * h2:64 * h2 + 32, sl],
                                              top[32 * h:32 * h + 32, sl])
                            nc.sync.dma_start(dst[64 * h2 + 32:64 * h2 + 64, sl],
                                              bot[32 * h:32 * h + 32, sl])

            # ---- phase B: v projection into [128, 4, 64] stationary tiles ----
            vos = []
            for s in range(ST):
                vo = bpool.tile([128, HC, 64], bf, tag=f"vo{s}", name=f"vo{s}")
                vos.append(vo)
                ps = sctpool.tile([128, 1024], f32, tag="sct", name="ps_v")
                for k in range(KT):
                    MM(f"vproj s{s} k{k}",
                        ps[:, :256],
                        lhsT=xslice(k, s * 128, 128),
                        rhs=wv[:, k, :],
                        start=(k == 0), stop=(k == KT - 1),
                    )
                nc.scalar.copy(vo.rearrange("p h w -> p (h w)"), ps[:, :256])

            # ---- attention with interleaved fillers ----
            gP = [bpool.tile([128, S], bf, tag=f"gP{g}", name=f"gP{g}") for g in range(2)]
            ag = [bpool.tile([128, S], bf, tag=f"ag{g}", name=f"ag{g}") for g in range(2)]

            def gate_group(g, half2):
                # half a gate-projection column-tile: 8 matmuls + tanh + affine
                def emit():
                    o = half2 * 512
                    sl = slice(o, o + 512)
                    ps = filpool.tile([128, 512], f32, tag="fil", name="ps_g")
                    for k in range(KT):
                        MM(f"gate g{g} o{o} k{k}",
                            ps[:, 0:512],
                            lhsT=wg[:, k, g * 128:(g + 1) * 128],
                            rhs=xslice(k, o, 512),
                            start=(k == 0), stop=(k == KT - 1),
                        )
                    th = wpool.tile([128, 512], bf, tag="th", name="th")
                    nc.scalar.activation(th, ps[:, 0:512], Act.Tanh, scale=0.5)
                    # sigmoid(x) = 0.5*tanh(x/2) + 0.5  (gpsimd: SBUF only)
                    nc.gpsimd.tensor_scalar(gP[g][:, sl], th, 0.5, 0.5, MUL, ADD)
                return emit

            def fil_tile(tail):
                # in-chunk fillers use the 1-bank pool; tail fillers run when
                # the scores pipeline is done, so they use the big sct pool
                if tail:
                    return sctpool.tile([128, 1024], f32, tag="sct", name="ps_tail")
                return filpool.tile([128, 512], f32, tag="fil", name="ps_fil")

            def recb_pair(c, rec_bf, recBs, g, cols=slice(0, 512), tail=False):
                # broadcast 1/den for the head pair g into ONE [128, 512]
                # tile: rows 0:64 <- rec_{2g}, rows 64:128 <- rec_{2g+1}.
                # K=64, M=64 per matmul; the two MMs fuse in the PE array.
                def emit():
                    ps = fil_tile(tail)
                    recBs[g] = ps
                    ksl = slice(0, 64) if g == 0 else slice(64, 128)
                    for h2 in range(2):
                        h = 2 * g + h2
                        MM(f"recB c{c} h{h}",
                            ps[64 * h2:64 * (h2 + 1), cols],
                            lhsT=hot[ksl, 128 * h:128 * h + 64],
                            rhs=rec_bf[ksl, cols],
                            start=True, stop=True,
                        )
                return emit

            def norm_pair(c, recBs, uus, g, half=None):
                # after recB: agh = uu * recB ; ag = agh * gate  (DVE + gpsimd)
                def emit():
                    csl = slice(c * SQ, (c + 1) * SQ)
                    if c == NCH - 1:
                        # uu is already gated: one mul to normalized output
                        cw = slice(half * 256, half * 256 + 256) if half is not None \
                            else slice(0, 512)
                        nc.vector.tensor_tensor(
                            ag[g][:, c * SQ + cw.start:c * SQ + cw.stop],
                            uus[g][:, cw],
                            recBs[g][:, cw], MUL)
                        return
                    agh = wpool.tile([128, SQ], f32, tag=f"agh{g}", name="agh", bufs=1)
                    nc.vector.tensor_tensor(agh, uus[g], recBs[g][:, 0:512], MUL)
                    nc.gpsimd.tensor_tensor(ag[g][:, csl], agh, gP[g][:, csl], MUL)
                return emit

            def oproj_half(c, st, n, tail=False):
                # one 512-col half of the output projection for row-tile st
                def emit():
                    s = (SQ // 128) * c + st
                    ps = fil_tile(tail)
                    for g in range(2):
                        MM(f"oproj c{c} st{st} n{n} g{g}",
                            ps[:, 0:512],
                            lhsT=ag[g][:, s * 128:(s + 1) * 128],
                            rhs=wo[:, g, n * 512:(n + 1) * 512],
                            start=(g == 0), stop=(g == 1),
                        )
                    ob = wpool.tile([128, 512], bf, tag="ob", name="ob", bufs=3)
                    # tail: split copies between ACT and DVE (both run the
                    # normalize chain; a single engine serializes the tail)
                    if tail and (st + n + c) % 2 == 1:
                        nc.scalar.copy(ob, ps[:, 0:512])
                    else:
                        nc.vector.tensor_copy(ob, ps[:, 0:512])
                    nc.sync.dma_start(out_d[s * 128:(s + 1) * 128,
                                            n * 512:(n + 1) * 512], ob)
                emit.deferred = (c == NCH - 2)
                return emit

            # filler queues: emitted between sk-steps of each chunk.
            # gate half2=c is needed by norm_pair(c) early in chunk c+1, so
            # halves 0,1 go in chunk 0 and 2,3 spread into chunks 1,2 (the
            # ACT-paced chunks have ~2 gate groups of PE slack each).
            fillers = {c: [] for c in range(NCH + 1)}
            fillers[0] = [gate_group(g, h2) for h2 in range(2) for g in range(2)]
            fillers[1] = [gate_group(g, 2) for g in range(2)]
            fillers[2] = [gate_group(g, 3) for g in range(2)]

            norm_state = {}
            for c in range(NCH):
                csl = slice(c * SQ, (c + 1) * SQ)
                pvp = [pvpool.tile([128, SQ], f32, tag=f"pvp{g}", name=f"pvp{g}")
                       for g in range(2)]
                denp = dnpool.tile([128, SQ], f32, tag="denp", name="denp")
                exs = {}

                def scores_step(sk):
                    for g in range(2):
                        sct = sctpool.tile([128, 1024], f32, tag="sct", name="sct")
                        for h2 in range(2):
                            MM(f"score c{c} sk{sk} g{g} h{h2}",
                                sct[:, h2 * 512:(h2 + 1) * 512],
                                lhsT=kR[g][64 * h2:64 * (h2 + 1), sk * 128:(sk + 1) * 128],
                                rhs=qR[g][64 * h2:64 * (h2 + 1), csl],
                                start=True, stop=True,
                            )
                        ex = expool.tile([128, 1024], bf, tag="ex", name="ex")
                        if (sk, g) in BT_UNITS:
                            # Schraudolph bit-trick on DVE: int16 bits of bf16
                            nc.vector.tensor_scalar(
                                ex.bitcast(i16), sct, BT_A, BT_B, MUL, ADD)
                        else:
                            nc.scalar.activation(ex, sct, Act.Exp, scale=scale)
                        exs[(sk, g)] = ex

                def pv_step(sk):
                    # pv pairs first, then all four den MMs adjacently so the
                    # M=1 quads fuse across the four PE column groups
                    exg = [exs.pop((sk, 0)), exs.pop((sk, 1))]
                    for g in range(2):
                        for h2 in range(2):
                            h = 2 * g + h2
                            MM(f"pv c{c} sk{sk} h{h}",
                                pvp[g][64 * h2:64 * (h2 + 1), :],
                                lhsT=vos[sk][:, h, :],
                                rhs=exg[g][:, h2 * 512:(h2 + 1) * 512],
                                start=(sk == 0), stop=(sk == ST - 1),
                            )
                    for h in range(HC):
                        g, h2 = divmod(h, 2)
                        MM(f"den c{c} sk{sk} h{h}",
                            denp[32 * h:32 * h + 1, :],
                            lhsT=ones32[:, 0:1],
                            rhs=exg[g][:, h2 * 512:(h2 + 1) * 512],
                            start=(sk == 0), stop=(sk == ST - 1),
                            tile_position=(0, 32 * h),
                        )

                fq = fillers[c]
                for sk in range(ST):
                    scores_step(sk)
                    if sk > 2:
                        pv_step(sk - 3)
                    if fq and sk >= 5:
                        fq.pop(0)()
                for sk in (ST - 3, ST - 2, ST - 1):
                    pv_step(sk)
                while fq:
                    fq.pop(0)()

                # ---- normalize: free pv banks early, recip off critical path ----
                last = (c == NCH - 1)
                rec_bf = wpool.tile([128, SQ], bf, tag="rec_bf", name="rec_bf", bufs=2)

                if last:
                    # latency-critical tail: dstack + two 256-wide DVE waves,
                    # dstack copies on ACT (idle in the tail)
                    dstack = wpool.tile([128, SQ], f32, tag="dstack", name="dstack", bufs=1)
                    nc.gpsimd.memset(dstack, 1.0)
                    for h in range(HC):
                        nc.scalar.copy(dstack[32 * h:32 * h + 1, :],
                                       denp[32 * h:32 * h + 1, :])
                    rec128 = wpool.tile([128, SQ], f32, tag="rec128", name="rec128", bufs=1)
                    nc.vector.reciprocal(out=rec128[:, 0:256],
                                         in_=dstack[:, 0:256])
                    nc.gpsimd.tensor_copy(rec_bf[:, 0:256], rec128[:, 0:256])
                    nc.vector.reciprocal(out=rec128[:, 256:512],
                                         in_=dstack[:, 256:512])
                    nc.scalar.copy(rec_bf[:, 256:512], rec128[:, 256:512])
                else:
                    # only 4x512 of the [128,512] den tile is live, so pack the
                    # four rows into [128,16] via SBUF DMA (off-engine), run the
                    # reciprocal on 16 elems/lane (~0.3us instead of 3.4us of
                    # DVE), and scatter back.  This unclogs the chunk-boundary
                    # DVE queue that was stalling the next chunk's bt-exps.
                    den_sb = wpool.tile([128, SQ], f32, tag="den_sb", name="den_sb", bufs=2)
                    nc.scalar.copy(den_sb, denp)
                    # free->partition reshape is illegal for SBUF-SBUF DMA, so
                    # bounce through scratch DRAM (linear; arbitrary APs)
                    den_scr = scrpool.tile([4, 512], f32, tag="den_scr", name="den_scr")
                    for h in range(HC):
                        nc.sync.dma_start(den_scr[h:h + 1, :],
                                          den_sb[32 * h:32 * h + 1, :])
                    dpack = wpool.tile([128, 16], f32, tag="dpack", name="dpack", bufs=2)
                    nc.sync.dma_start(
                        dpack, den_scr.rearrange("h (i j) -> (h i) j", j=16))
                    rpack = wpool.tile([128, 16], f32, tag="rpack", name="rpack", bufs=2)
                    nc.vector.reciprocal(out=rpack, in_=dpack)
                    rpack_bf = wpool.tile([128, 16], bf, tag="rpack_bf", name="rpack_bf", bufs=2)
                    nc.gpsimd.tensor_copy(rpack_bf, rpack)
                    rec_scr = scrpool.tile([128, 16], bf, tag="rec_scr", name="rec_scr")
                    nc.sync.dma_start(rec_scr, rpack_bf)
                    rs4 = rec_scr.rearrange("(h i) j -> h (i j)", h=4)
                    for h in range(HC):
                        nc.sync.dma_start(rec_bf[32 * h:32 * h + 1, :],
                                          rs4[h:h + 1, :])
                uus = []
                for g in range(2):
                    uu = wpool.tile([128, SQ], f32, tag=f"uu{g}", name=f"uu{g}", bufs=1)
                    if last:
                        # gated u: fold the gate multiply in here so it
                        # overlaps the reciprocal instead of trailing recB
                        nc.vector.tensor_tensor(uu, pvp[g], gP[g][:, csl], MUL)
                    else:
                        nc.vector.tensor_copy(uu, pvp[g])
                    uus.append(uu)

                recBs = [None, None]
                if c == NCH - 1:
                    # two waves: [recb+norm(A), st0, st1] then [recb+norm(B), ...]
                    for half in range(2):
                        cw = slice(half * 256, half * 256 + 256)
                        for g in range(2):
                            fillers[c + 1].append(
                                recb_pair(c, rec_bf, recBs, g, cw, tail=True))
                            fillers[c + 1].append(
                                norm_pair(c, recBs, uus, g, half))
                        for st in (2 * half, 2 * half + 1):
                            fillers[c + 1].append(oproj_half(c, st, 0, tail=True))
                            fillers[c + 1].append(oproj_half(c, st, 1, tail=True))
                else:
                    for g in range(2):
                        fillers[c + 1].append(recb_pair(c, rec_bf, recBs, g))
                        fillers[c + 1].append(norm_pair(c, recBs, uus, g))
                    for st in range(SQ // 128):
                        # the last chunk's predecessor defers three oproj
                        # tiles into the tail to cover the final normalize
                        tail = (c == NCH - 2)
                        dst = c + 2 if tail else c + 1
                        fillers[dst].append(oproj_half(c, st, 0, tail=tail))
                        fillers[dst].append(oproj_half(c, st, 1, tail=tail))

            # drain: deferred oproj halves lead (they cover the final
            # normalize/reciprocal chain), interleaved 2:1 with the waves
            deferred = [f for f in fillers[NCH] if getattr(f, "deferred", False)]
            waves = [f for f in fillers[NCH] if not getattr(f, "deferred", False)]
            drain = []
            while deferred or waves:
                for _ in range(2):
                    if deferred:
                        drain.append(deferred.pop(0))
                if waves:
                    drain.append(waves.pop(0))
            for f in drain:
                f()

    return nc


def _host_inputs(x, w_qkv, w_gate, w_out):
    """Build the 8 per-core input maps (all device tensors bf16)."""
    bf = ml_dtypes.bfloat16
    x = np.asarray(x, dtype=np.float32)
    w_qkv = np.asarray(w_qkv, dtype=np.float32)
    w_gate = np.asarray(w_gate, dtype=np.float32)
    w_out = np.asarray(w_out, dtype=np.float32)

    inv = 1.0 / (ROPE_THETA ** (np.arange(0, D, 2, dtype=np.float64) / D))   # [32]
    ang = np.arange(S, dtype=np.float64)[None, :] * inv[:, None]             # [32, S]
    cs = np.tile(np.cos(ang), (4, 1)).astype(bf)                             # [128, S]
    sn = np.tile(np.sin(ang), (4, 1)).astype(bf)

    hot = np.zeros((128, 512), dtype=bf)
    for h in range(HC):
        hot[32 * h, 128 * h:128 * (h + 1)] = 1.0

    wq = w_qkv[:, 0:E]
    wk = w_qkv[:, E:2 * E]
    wvv = w_qkv[:, 2 * E:3 * E]

    in_maps = []
    for c in range(NCORES):
        b = c // 4
        hs = HC * (c % 4)
        cols_ev = np.concatenate([(hs + h) * 64 + np.arange(0, 64, 2) for h in range(HC)])
        cols_od = cols_ev + 1
        wqk_p = np.concatenate(
            [wq[:, cols_ev], wq[:, cols_od], wk[:, cols_ev], wk[:, cols_od]], axis=1)
        vcols = np.concatenate([(hs + h) * 64 + np.arange(64) for h in range(HC)])
        wo_p = w_out[vcols, :].reshape(2, 128, E).transpose(1, 0, 2)
        xT = np.ascontiguousarray(x[b].T).astype(bf)
        m = {
            "wv": np.ascontiguousarray(wvv[:, vcols]).astype(bf),
            "wg": np.ascontiguousarray(w_gate[:, vcols]).astype(bf),
            "wo": np.ascontiguousarray(wo_p).astype(bf),
            "cs": cs,
            "sn": sn,
            "hot": hot,
        }
        for k in range(KT):
            m[f"wqka{k}"] = np.ascontiguousarray(wqk_p[k * 128:(k + 1) * 128, 0:256]).astype(bf)
            m[f"wqkb{k}"] = np.ascontiguousarray(wqk_p[k * 128:(k + 1) * 128, 256:512]).astype(bf)
            m[f"xn{k}"] = np.ascontiguousarray(xT[k * 128:(k + 1) * 128, 0:1024])
            m[f"xm{k}"] = np.ascontiguousarray(xT[k * 128:(k + 1) * 128, 1024:2048])
        in_maps.append(m)
    return in_maps


def kernel(x, w_qkv, w_gate, w_out, b_out, n_heads):
    global LAST_RESULTS
    assert int(n_heads) == H
    x = np.asarray(x)
    assert x.shape == (B, S, E)

    from concourse.bass_utils import run_bass_kernel_spmd

    _install_birfix()
    if "nc" not in _CACHE:
        _CACHE["nc"] = _build_nc()
    nc = _CACHE["nc"]

    in_maps = _host_inputs(x, w_qkv, w_gate, w_out)
    import os
    trace = bool(int(os.environ.get("KERNEL_TRACE", "0")))
    tmpdir = os.environ.get("KERNEL_TRACE_DIR") if trace else None
    res = run_bass_kernel_spmd(nc, in_maps, list(range(NCORES)), trace=trace,
                               tmpdir=tmpdir)
    LAST_RESULTS = res

    out = np.zeros((B, S, E), dtype=np.float32)
    for c in range(NCORES):
        out[c // 4] += np.asarray(res.results[c]["out"], dtype=np.float32)
    out += np.asarray(b_out, dtype=np.float32)[None, None, :]
    return out
